# revision 17
# baseline (speedup 1.0000x reference)
"""Single-head attention (no causal mask) on 8 Trainium2 NeuronCores.

Problem: inputs [32, 2048, 64], Wq/Wk/Wv [64, 64] (nn.Linear style, out = x @ W.T).
  q = x @ Wq^T ; k = x @ Wk^T ; v = x @ Wv^T
  out = softmax(q @ k^T / 8) @ v          # no causal mask in the reference

Sharding: data-parallel over batch — 4 batch images per core, weights replicated.

Per-core design (v5):
  - Host pre-transposes x to xT [64, 2048] bf16; weights host-transposed to
    bf16, 1/8 folded into Wq, Wq/Wk duplicated column-wise ([W|W], 64x128) so
    the projection materializes qT/kT twice: partitions 0-63 and 64-127 hold
    identical copies. Even k-chunks' score matmuls read the low half (PE
    row-tile T0), odd chunks the high half (T8); the two 64-row tiles stream
    concurrently (K=64 only fills half the array). All PE operands are bf16.
  - Warmup burst of dependency-free K=128 matmuls at t=0 flips the PE HAM
    clock gate to 8/8 (2.4 GHz) before the real pipeline starts.
  - Flat chunk-step software pipeline: step s emits scores+exp for chunk s and
    the AV accumulation for chunk s-LAG, crossing batch boundaries without
    phase barriers. Projections for batch b+1 are spread into fixed chunk
    slots; U^T is evacuated in 512-wide quarters immediately after the last
    AV matmul of each region (alternating ScalarE/VectorE) so the next
    batch's accumulation never waits.
  - exp split between ScalarE (table exp) and VectorE (bf16 Schraudolph:
    bitcast(int16(x*184.665 + 16250.4)); ~+-3% on affected slices, measured
    end-to-end error ~8e-3 absmax vs 2e-2 budget).
  - U^T [65, 2048] accumulated over k-chunks with lhsT = [v | 1] (row 64 =
    softmax denominator); divide + [h,s]->[s,h] transpose on host.
"""

import math
from contextlib import ExitStack

import numpy as np

import concourse.bass as bass
import concourse.mybir as mybir
import concourse.tile as tile
from concourse import bacc
from concourse.bass import ds, ts
from concourse.bass_utils import run_bass_kernel_spmd

F32 = mybir.dt.float32
BF16 = mybir.dt.bfloat16
I16 = mybir.dt.int16
EXP = mybir.ActivationFunctionType.Exp
MULT = mybir.AluOpType.mult
ADD = mybir.AluOpType.add

B, S, E, H = 32, 2048, 64, 64
NCORES = 8
BC = B // NCORES  # batches per core
NCH = S // 128  # k-chunks per batch
QJ = 512  # PSUM bank / exp instruction width along q
NQJ = S // QJ

# Schraudolph bf16 exp: bitcast(int16(x*SCHR_A + SCHR_B)) ~= exp(x)
SCHR_C = 5.6
SCHR_A = 128.0 / math.log(2.0)
SCHR_B = 127.0 * 128.0 - SCHR_C

# (c, j) exp instructions computed on VectorE via Schraudolph (rest: ScalarE).
# j indexes the four 512-wide q slices; DVE takes the upper q half of chunks
# 1..13 (26 of 64 slices).
EXP_DVE = {(c, j) for c in range(1, NCH - 2) for j in (2, 3)}
LAG = 3  # AV trails scores by this many chunk-steps
WARMUP_MMS = 24


def build_nc():
    nc = bacc.Bacc("TRN2", target_bir_lowering=False, debug=False)

    xt_d = nc.dram_tensor("xt", [BC, E, S], BF16, kind="ExternalInput").ap()
    wq_d = nc.dram_tensor("wq", [E, 2 * H], BF16, kind="ExternalInput").ap()
    wk_d = nc.dram_tensor("wk", [E, 2 * H], BF16, kind="ExternalInput").ap()
    wv_d = nc.dram_tensor("wv", [E, H], BF16, kind="ExternalInput").ap()
    out_d = nc.dram_tensor("out", [BC, H + 1, S], F32, kind="ExternalOutput").ap()

    ctx = ExitStack()
    with tile.TileContext(nc) as tc:
        with ctx:
            const = ctx.enter_context(tc.tile_pool(name="const", bufs=1))
            xt_pool = ctx.enter_context(tc.tile_pool(name="xt", bufs=3))
            qk_pool = ctx.enter_context(tc.tile_pool(name="qk", bufs=4))
            va_pool = ctx.enter_context(tc.tile_pool(name="va", bufs=2))
            ex_pool = ctx.enter_context(tc.tile_pool(name="ex", bufs=30))
            ut_sb_pool = ctx.enter_context(tc.tile_pool(name="utsb", bufs=4))
            ps_a = ctx.enter_context(tc.tile_pool(name="ps_a", bufs=4, space="PSUM"))
            ps_u = ctx.enter_context(tc.tile_pool(name="ps_u", bufs=1, space="PSUM"))

            ones = const.tile([128, NCH], F32, tag="ones")
            nc.gpsimd.memset(ones[:], 1.0)
            # scratch operands for the warmup burst (results discarded)
            scr_w = const.tile([128, 128], BF16, tag="scr_w")
            scr_x = const.tile([128, QJ], BF16, tag="scr_x")
            nc.gpsimd.memset(scr_w[:], 0.0)
            nc.gpsimd.memset(scr_x[:], 0.0)
            wq_s = const.tile([E, 2 * H], BF16, tag="wq")
            wk_s = const.tile([E, 2 * H], BF16, tag="wk")
            wv_s = const.tile([E, H], BF16, tag="wv")
            nc.sync.dma_start(wq_s[:], wq_d)
            nc.sync.dma_start(wk_s[:], wk_d)
            nc.sync.dma_start(wv_s[:], wv_d)

            # HAM warmup: dependency-free back-to-back full-array (K=128)
            # matmuls; the clock gate needs a full 4096-cycle window of
            # sustained activity to flip to 8/8.
            warm = ps_a.tile([128, QJ], F32, tag="ps")
            for _ in range(WARMUP_MMS):
                nc.tensor.matmul(
                    warm[:, :], scr_w[:], scr_x[:], start=True, stop=True
                )

            def load_xt(b):
                xt_t = xt_pool.tile([E, S], BF16, tag="xt")
                nc.sync.dma_start(xt_t[:], xt_d[b])
                return xt_t

            def proj_qk(xt_t, w_s, which):
                """[128, S] bf16: rows 0-63 and 64-127 both hold W.T @ xT."""
                dst = qk_pool.tile([128, S], BF16, tag="qk")
                for j in range(NQJ):
                    pp = ps_a.tile([128, QJ], F32, tag="ps")
                    nc.tensor.matmul(
                        pp[:, :],
                        w_s[:],
                        xt_t[:, ts(j, QJ)],
                        start=True,
                        stop=True,
                    )
                    if (j + which) % 2 == 0:
                        nc.scalar.copy(dst[:, ts(j, QJ)], pp[:])
                    else:
                        nc.vector.tensor_copy(dst[:, ts(j, QJ)], pp[:])
                return dst

            def proj_v(xt_t):
                """va [128, NCH*65] bf16: per chunk, 64 v-cols + ones col."""
                va = va_pool.tile([128, NCH * 65], BF16, tag="va")
                va_v = va[:].rearrange("p (c w) -> p c w", w=65)
                nc.vector.tensor_copy(
                    va_v[:, :, 64:65],
                    ones[:].rearrange("p (c w) -> p c w", w=1),
                )
                for half in range(2):
                    vp = ps_a.tile([128, QJ], F32, tag="ps")
                    for c in range(NCH // 2):
                        nc.tensor.matmul(
                            vp[:, ts(c, 64)],
                            xt_t[:, ds(half * 1024 + c * 128, 128)],
                            wv_s[:],
                            start=True,
                            stop=True,
                        )
                    src = vp[:].rearrange("p (c w) -> p c w", w=64)
                    dst = va_v[:, ds(half * (NCH // 2), NCH // 2), 0:64]
                    if half == 0:
                        nc.scalar.copy(dst, src)
                    else:
                        nc.vector.tensor_copy(dst, src)
                return va

            def scores_chunk(qT, kT, c):
                """exp(scores^T) for k-chunk c -> 4 ex tiles [128, QJ] bf16."""
                half = (c % 2) * 64
                exs = []
                for j in range(NQJ):
                    sct = ps_a.tile([128, QJ], F32, tag="ps")
                    nc.tensor.matmul(
                        sct[:, :],
                        kT[:][ds(half, 64), ds(c * 128, 128)],
                        qT[:][ds(half, 64), ts(j, QJ)],
                        start=True,
                        stop=True,
                    )
                    ex = ex_pool.tile([128, QJ], BF16, tag="ex")
                    if (c, j) in EXP_DVE:
                        nc.vector.tensor_scalar(
                            ex[:].bitcast(I16), sct[:], SCHR_A, SCHR_B, MULT, ADD
                        )
                    else:
                        nc.scalar.activation(ex[:], sct[:], EXP)
                    exs.append(ex)
                return exs

            def av_chunk(ut_ps, ut_sb, b, va, exs_c, c):
                """U^T += va.T @ ex; on the last chunk, evacuate each 512-wide
                quarter right after its final matmul and DMA out per half."""
                va_v = va[:].rearrange("p (c w) -> p c w", w=65)
                for j in range(NQJ):
                    nc.tensor.matmul(
                        ut_ps[0 : H + 1, ts(j, QJ)],
                        va_v[:, c, :],
                        exs_c[j][:, :],
                        start=(c == 0),
                        stop=(c == NCH - 1),
                    )
                    if c == NCH - 1:
                        if j % 2 == 0:
                            nc.scalar.copy(
                                ut_sb[:, ts(j, QJ)], ut_ps[0 : H + 1, ts(j, QJ)]
                            )
                        else:
                            nc.vector.tensor_copy(
                                ut_sb[:, ts(j, QJ)], ut_ps[0 : H + 1, ts(j, QJ)]
                            )
                            nc.sync.dma_start(
                                out_d[b][:, ds((j - 1) * QJ, 2 * QJ)],
                                ut_sb[:, ds((j - 1) * QJ, 2 * QJ)],
                            )

            # prologue: batch 0 projections, xt prefetch for 0 and 1
            xts = {0: load_xt(0), 1: load_xt(1)}
            qTs = {0: proj_qk(xts[0], wq_s, 0)}
            kTs = {0: proj_qk(xts[0], wk_s, 1)}
            vas = {0: proj_v(xts[0])}

            exs_all = {}  # (b, c) -> [ex_j0..ex_j3]
            ut_tiles = {}
            for s in range(BC * NCH + LAG):
                if s < BC * NCH:
                    b, c = divmod(s, NCH)
                    exs_all[(b, c)] = scores_chunk(qTs[b], kTs[b], c)
                    if b + 1 < BC:
                        if c == 3:
                            qTs[b + 1] = proj_qk(xts[b + 1], wq_s, 0)
                        elif c == 7:
                            kTs[b + 1] = proj_qk(xts[b + 1], wk_s, 1)
                        elif c == 11:
                            vas[b + 1] = proj_v(xts[b + 1])
                    if c == 8 and b + 2 < BC:
                        xts[b + 2] = load_xt(b + 2)
                    if c == NCH - 1:
                        xts.pop(b, None)
                av = s - LAG
                if av >= 0:
                    ba, ca = divmod(av, NCH)
                    if ca == 0:
                        ut_ps = ps_u.tile([H + 1, S], F32, tag="utp")
                        ut_sb = ut_sb_pool.tile([H + 1, S], F32, tag="ut")
                        ut_tiles[ba] = (ut_ps, ut_sb)
                    av_chunk(*ut_tiles[ba], ba, vas[ba], exs_all.pop((ba, ca)), ca)
                    if ca == NCH - 1:
                        ut_tiles.pop(ba)

    nc.compile()
    return nc


_NC = None


def _get_nc():
    global _NC
    if _NC is None:
        _NC = build_nc()
    return _NC


def _in_maps(inputs, Wq, Wk, Wv):
    import ml_dtypes

    bf = ml_dtypes.bfloat16
    xt = np.ascontiguousarray(
        np.transpose(inputs, (0, 2, 1)).astype(bf)
    )
    wq1 = Wq.T.astype(np.float32) / np.float32(np.sqrt(H))
    wq = np.ascontiguousarray(np.concatenate([wq1, wq1], axis=1).astype(bf))
    wk1 = Wk.T.astype(np.float32)
    wk = np.ascontiguousarray(np.concatenate([wk1, wk1], axis=1).astype(bf))
    wv = np.ascontiguousarray(Wv.T.astype(bf))
    return [
        {"xt": xt[c * BC : (c + 1) * BC], "wq": wq, "wk": wk, "wv": wv}
        for c in range(NCORES)
    ]


def run(inputs, Wq, Wk, Wv, **spmd_kwargs):
    nc = _get_nc()
    res = run_bass_kernel_spmd(
        nc, _in_maps(inputs, Wq, Wk, Wv), core_ids=list(range(NCORES)), **spmd_kwargs
    )
    # Each core returns U^T [BC, 65, S]; row 64 is the softmax denominator.
    outs = []
    for r in res.results:
        ut = r["out"]
        outs.append(
            np.transpose(ut[:, :H, :] / ut[:, H : H + 1, :], (0, 2, 1))
        )
    return np.ascontiguousarray(np.concatenate(outs, 0), dtype=np.float32), res


def kernel(inputs, Wq, Wk, Wv):
    out, _ = run(inputs, Wq, Wk, Wv)
    return out


# revision 18
# speedup vs baseline: 1.6078x; 1.6078x over previous
"""Single-head attention (no causal mask) on 8 Trainium2 NeuronCores.

Problem: inputs [32, 2048, 64], Wq/Wk/Wv [64, 64] (nn.Linear style, out = x @ W.T).
  q = x @ Wq^T ; k = x @ Wk^T ; v = x @ Wv^T
  out = softmax(q @ k^T / 8) @ v          # no causal mask in the reference

Sharding: data-parallel over batch — 4 batch images per core, weights replicated.

Per-core design (v6):
  - Host pre-transposes x to xT [64, 2048] bf16; weights host-transposed to
    bf16, 1/8 folded into Wq, Wq/Wk duplicated column-wise ([W|W], 64x128) so
    the projection materializes qT/kT twice (partitions 0-63 / 64-127). Even
    k-chunks' score matmuls read the low half (PE row-tile T0), odd chunks the
    high half (T8); the two 64-row tiles stream concurrently.
  - The PE clock gate (HAM) is bistable: once the PE streams without gaps it
    runs at 2.4 GHz, with gaps it sticks at 1.2 GHz. A K=128 warmup burst
    flips it warm at t=0; each batch is processed in two q-half passes so the
    U^T accumulator needs only 2 PSUM banks, freeing 6 banks for a
    3-chunk-deep score pipeline that keeps PE dependencies pre-satisfied.
  - Flat step pipeline over (batch, q-half, chunk): step s emits scores+exp
    for chunk s and AV for chunk s-LAG. Projections for batch b+1 are spread
    into fixed slots; U^T is evacuated in 512-wide quarters inline after the
    final AV matmul of each region (ScalarE/VectorE alternating).
  - exp split between ScalarE (table exp) and VectorE (bf16 Schraudolph:
    bitcast(int16(x*184.665 + 16250.4)); measured end-to-end error ~8e-3
    absmax vs the 2e-2 budget).
  - U^T [65, S] accumulated with lhsT = [v | 1] (row 64 = softmax
    denominator); divide + [h,s]->[s,h] transpose on host.
"""

import math
from contextlib import ExitStack

import numpy as np

import concourse.bass as bass
import concourse.mybir as mybir
import concourse.tile as tile
from concourse import bacc
from concourse.bass import ds, ts
from concourse.bass_utils import run_bass_kernel_spmd

F32 = mybir.dt.float32
BF16 = mybir.dt.bfloat16
I16 = mybir.dt.int16
EXP = mybir.ActivationFunctionType.Exp
MULT = mybir.AluOpType.mult
ADD = mybir.AluOpType.add

B, S, E, H = 32, 2048, 64, 64
NCORES = 8
BC = B // NCORES  # batches per core
NCH = S // 128  # k-chunks per batch
QH = 1024  # q-half width (exp instruction width)
NHALF = S // QH

# Schraudolph bf16 exp: bitcast(int16(x*SCHR_A + SCHR_B)) ~= exp(x)
SCHR_C = 5.6
SCHR_A = 128.0 / math.log(2.0)
SCHR_B = 127.0 * 128.0 - SCHR_C

# (half, c) exp instructions on VectorE via Schraudolph (rest: ScalarE).
EXP_DVE = {(1, c) for c in range(1, NCH - 2)}
LAG = 3  # AV trails scores by this many steps
WARMUP_MMS = 24


def build_nc():
    nc = bacc.Bacc("TRN2", target_bir_lowering=False, debug=False)

    xt_d = nc.dram_tensor("xt", [BC, E, S], BF16, kind="ExternalInput").ap()
    wq_d = nc.dram_tensor("wq", [E, 2 * H], BF16, kind="ExternalInput").ap()
    wk_d = nc.dram_tensor("wk", [E, 2 * H], BF16, kind="ExternalInput").ap()
    wv_d = nc.dram_tensor("wv", [E, H], BF16, kind="ExternalInput").ap()
    out_d = nc.dram_tensor("out", [BC, H + 1, S], F32, kind="ExternalOutput").ap()

    ctx = ExitStack()
    with tile.TileContext(nc) as tc:
        with ctx:
            const = ctx.enter_context(tc.tile_pool(name="const", bufs=1))
            xt_pool = ctx.enter_context(tc.tile_pool(name="xt", bufs=3))
            qk_pool = ctx.enter_context(tc.tile_pool(name="qk", bufs=4))
            va_pool = ctx.enter_context(tc.tile_pool(name="va", bufs=2))
            ex_pool = ctx.enter_context(tc.tile_pool(name="ex", bufs=12))
            ut_sb_pool = ctx.enter_context(tc.tile_pool(name="utsb", bufs=4))
            ps_a = ctx.enter_context(tc.tile_pool(name="ps_a", bufs=3, space="PSUM"))
            ps_u = ctx.enter_context(tc.tile_pool(name="ps_u", bufs=1, space="PSUM"))

            ones = const.tile([128, NCH], F32, tag="ones")
            nc.gpsimd.memset(ones[:], 1.0)
            # scratch operands for the warmup burst (results discarded)
            scr_w = const.tile([128, 128], BF16, tag="scr_w")
            scr_x = const.tile([128, 512], BF16, tag="scr_x")
            nc.gpsimd.memset(scr_w[:], 0.0)
            nc.gpsimd.memset(scr_x[:], 0.0)
            wq_s = const.tile([E, 2 * H], BF16, tag="wq")
            wk_s = const.tile([E, 2 * H], BF16, tag="wk")
            wv_s = const.tile([E, H], BF16, tag="wv")
            nc.sync.dma_start(wq_s[:], wq_d)
            nc.sync.dma_start(wk_s[:], wk_d)
            nc.sync.dma_start(wv_s[:], wv_d)

            # HAM warmup: dependency-free back-to-back full-array matmuls
            warm = ps_a.tile([128, QH], F32, tag="ps")
            for _ in range(WARMUP_MMS):
                nc.tensor.matmul(
                    warm[:, 0:512], scr_w[:], scr_x[:], start=True, stop=True
                )

            def load_xt(b):
                xt_t = xt_pool.tile([E, S], BF16, tag="xt")
                nc.sync.dma_start(xt_t[:], xt_d[b])
                return xt_t

            def proj_qk(xt_t, w_s, which):
                """[128, S] bf16: rows 0-63 and 64-127 both hold W.T @ xT."""
                dst = qk_pool.tile([128, S], BF16, tag="qk")
                for h2 in range(S // QH):
                    pp = ps_a.tile([128, QH], F32, tag="ps")
                    for j in range(QH // 512):
                        nc.tensor.matmul(
                            pp[:, ts(j, 512)],
                            w_s[:],
                            xt_t[:, ds(h2 * QH + j * 512, 512)],
                            start=True,
                            stop=True,
                        )
                    if (h2 + which) % 2 == 0:
                        nc.scalar.copy(dst[:, ds(h2 * QH, QH)], pp[:])
                    else:
                        nc.vector.tensor_copy(dst[:, ds(h2 * QH, QH)], pp[:])
                return dst

            def proj_v(xt_t):
                """va [128, NCH*65] bf16: per chunk, 64 v-cols + ones col."""
                va = va_pool.tile([128, NCH * 65], BF16, tag="va")
                va_v = va[:].rearrange("p (c w) -> p c w", w=65)
                nc.vector.tensor_copy(
                    va_v[:, :, 64:65],
                    ones[:].rearrange("p (c w) -> p c w", w=1),
                )
                vp = ps_a.tile([128, QH], F32, tag="ps")
                for c in range(NCH):
                    nc.tensor.matmul(
                        vp[:, ts(c, 64)],
                        xt_t[:, ts(c, 128)],
                        wv_s[:],
                        start=True,
                        stop=True,
                    )
                src = vp[:].rearrange("p (c w) -> p c w", w=64)
                nhalf = NCH // 2
                nc.scalar.copy(va_v[:, 0:nhalf, 0:64], src[:, 0:nhalf, :])
                nc.vector.tensor_copy(
                    va_v[:, nhalf:NCH, 0:64], src[:, nhalf:NCH, :]
                )
                return va

            def scores_chunk(qT, kT, half, c):
                """exp(scores^T) for k-chunk c, q-half -> ex [128, QH] bf16."""
                ksel = (c % 2) * 64
                sct = ps_a.tile([128, QH], F32, tag="ps")
                for j in range(QH // 512):
                    nc.tensor.matmul(
                        sct[:, ts(j, 512)],
                        kT[:][ds(ksel, 64), ds(c * 128, 128)],
                        qT[:][ds(ksel, 64), ds(half * QH + j * 512, 512)],
                        start=True,
                        stop=True,
                    )
                ex = ex_pool.tile([128, QH], BF16, tag="ex")
                if (half, c) in EXP_DVE:
                    nc.vector.tensor_scalar(
                        ex[:].bitcast(I16), sct[:], SCHR_A, SCHR_B, MULT, ADD
                    )
                else:
                    nc.scalar.activation(ex[:], sct[:], EXP)
                return ex

            def av_chunk(ut_ps, ut_sb, b, half, va, ex, c):
                """U^T half += va.T @ ex; inline quarter-evac on last chunk."""
                va_v = va[:].rearrange("p (c w) -> p c w", w=65)
                for j in range(QH // 512):
                    nc.tensor.matmul(
                        ut_ps[0 : H + 1, ts(j, 512)],
                        va_v[:, c, :],
                        ex[:, ts(j, 512)],
                        start=(c == 0),
                        stop=(c == NCH - 1),
                    )
                    if c == NCH - 1:
                        if j == 0:
                            nc.scalar.copy(
                                ut_sb[:, ds(half * QH, 512)],
                                ut_ps[0 : H + 1, 0:512],
                            )
                        else:
                            nc.vector.tensor_copy(
                                ut_sb[:, ds(half * QH + 512, 512)],
                                ut_ps[0 : H + 1, 512:QH],
                            )
                            nc.sync.dma_start(
                                out_d[b][:, ds(half * QH, QH)],
                                ut_sb[:, ds(half * QH, QH)],
                            )

            # prologue: batch 0 projections, xt prefetch for 0 and 1
            xts = {0: load_xt(0), 1: load_xt(1)}
            qTs = {0: proj_qk(xts[0], wq_s, 0)}
            kTs = {0: proj_qk(xts[0], wk_s, 1)}
            vas = {0: proj_v(xts[0])}

            NSTEP = NHALF * NCH  # steps per batch
            exs_all = {}
            ut_cur = {}
            ut_sbs = {}
            for s in range(BC * NSTEP + LAG):
                if s < BC * NSTEP:
                    b, r = divmod(s, NSTEP)
                    half, c = divmod(r, NCH)
                    exs_all[(b, half, c)] = scores_chunk(qTs[b], kTs[b], half, c)
                    if b + 1 < BC and half == 0:
                        if c == 3:
                            qTs[b + 1] = proj_qk(xts[b + 1], wq_s, 0)
                        elif c == 9:
                            kTs[b + 1] = proj_qk(xts[b + 1], wk_s, 1)
                    if b + 1 < BC and half == 1:
                        if c == 3:
                            vas[b + 1] = proj_v(xts[b + 1])
                        if c == 8 and b + 2 < BC:
                            xts[b + 2] = load_xt(b + 2)
                        if c == NCH - 1:
                            xts.pop(b, None)
                av = s - LAG
                if av >= 0:
                    ba, ra = divmod(av, NSTEP)
                    ha, ca = divmod(ra, NCH)
                    if ca == 0:
                        ut_ps = ps_u.tile([H + 1, QH], F32, tag="utp")
                        ut_cur[(ba, ha)] = ut_ps
                        if ha == 0:
                            ut_sb = ut_sb_pool.tile([H + 1, S], F32, tag="ut")
                            ut_sbs[ba] = ut_sb
                    av_chunk(
                        ut_cur[(ba, ha)],
                        ut_sbs[ba],
                        ba,
                        ha,
                        vas[ba],
                        exs_all.pop((ba, ha, ca)),
                        ca,
                    )
                    if ca == NCH - 1:
                        ut_cur.pop((ba, ha))
                        if ha == NHALF - 1:
                            ut_sbs.pop(ba)

    nc.compile()
    return nc


_NC = None


def _get_nc():
    global _NC
    if _NC is None:
        _NC = build_nc()
    return _NC


def _in_maps(inputs, Wq, Wk, Wv):
    import ml_dtypes

    bf = ml_dtypes.bfloat16
    xt = np.ascontiguousarray(np.transpose(inputs, (0, 2, 1)).astype(bf))
    wq1 = Wq.T.astype(np.float32) / np.float32(np.sqrt(H))
    wq = np.ascontiguousarray(np.concatenate([wq1, wq1], axis=1).astype(bf))
    wk1 = Wk.T.astype(np.float32)
    wk = np.ascontiguousarray(np.concatenate([wk1, wk1], axis=1).astype(bf))
    wv = np.ascontiguousarray(Wv.T.astype(bf))
    return [
        {"xt": xt[c * BC : (c + 1) * BC], "wq": wq, "wk": wk, "wv": wv}
        for c in range(NCORES)
    ]


def run(inputs, Wq, Wk, Wv, **spmd_kwargs):
    nc = _get_nc()
    res = run_bass_kernel_spmd(
        nc, _in_maps(inputs, Wq, Wk, Wv), core_ids=list(range(NCORES)), **spmd_kwargs
    )
    # Each core returns U^T [BC, 65, S]; row 64 is the softmax denominator.
    outs = []
    for r in res.results:
        ut = r["out"]
        outs.append(
            np.transpose(ut[:, :H, :] / ut[:, H : H + 1, :], (0, 2, 1))
        )
    return np.ascontiguousarray(np.concatenate(outs, 0), dtype=np.float32), res


def kernel(inputs, Wq, Wk, Wv):
    out, _ = run(inputs, Wq, Wk, Wv)
    return out
